# revision 15
# baseline (speedup 1.0000x reference)
"""Trainium2 Bass kernel: 7x7 valid cross-correlation + bias on a 4096x4096 f32 image.

Formulation: banded matmul on the TensorEngine.
  out[r, c] = sum_{di,dj} w[di,dj] * x[r+di, c+dj]
For an output row-strip of M=122 rows starting at r0, using K=128 input rows:
  out[r0+m, c] = sum_k A_dj[k, m] * x[r0+k, c+dj]   summed over dj=0..6
where A_dj[k, m] = w[k-m, dj] for 0 <= k-m < 7 (a banded [128, 122] matrix,
precomputed on host from the 49 kernel weights). The 7 dj-terms accumulate
into one PSUM bank via shifted column slices of the same SBUF rhs tile.

All matmul operands are fp16 (1 cycle/row on the PE vs fp32's 4), PSUM
accumulates fp32, output staged to SBUF as fp16 and upcast on the host.

DMA strategy: each dma_start costs ~1.2us of sequencer time regardless of
size, and PE stalls reset the p-state ramp (2.4GHz only after 3us of
continuous execution). So the host pre-arranges the input strip-major as
xst[p, s, c] = x[122*s + p, c0 + c] and the kernel loads it in a few large
chunk DMAs on the SP queue; outputs are written strip-major to
outt[m, s, c] = out[122*s + m, c0 + c] in one batched DMA per weight-group
on the Activation queue. The PE then streams matmuls back-to-back.

Sharding: output columns are split across the 8 cores (512 cols/core);
each core processes all 4090 output rows. Kernel + bias replicated.
"""

import numpy as np

H, W = 4096, 4096
KH, KW = 7, 7
OH, OW = H - KH + 1, W - KW + 1  # 4090, 4090
N_CORES = 8
CW = 512               # output columns per core
IW = CW + KW - 1       # input columns per core (518)
STRIP = 122            # output rows per strip (K = STRIP + KH - 1 = 128)
MB = 128               # stationary block columns (M padded 122 -> 128)
N_STRIPS = (OH + STRIP - 1) // STRIP  # 34 (last strip M=64, K=70)
G = 4                  # strips per weight-group (PSUM pipelining)
CHUNK_SIZES = (4, 6, 8, 8, 8)  # input DMA chunks; chunk0 = exactly group0
OUT_BATCHES = (8, 8, 8, 6, 2, 1, 1)  # strips per output DMA: big early, tiny at the end
N_WARM = 8             # junk matmuls at t=0 to ramp the PE p-state clock

_cache = {}


def _chunks():
    s0 = 0
    out = []
    for ns in CHUNK_SIZES:
        ns = min(ns, N_STRIPS - s0)
        if ns <= 0:
            break
        out.append((s0, ns))
        s0 += ns
    assert s0 == N_STRIPS, (s0, N_STRIPS)
    return out


def _build_nc():
    import concourse.bacc as bacc
    import concourse.mybir as mybir
    from concourse.tile import TileContext

    f16 = mybir.dt.float16
    f32 = mybir.dt.float32

    n_chunks = len(_chunks())

    nc = bacc.Bacc("TRN2", target_bir_lowering=False, debug=False)
    xst = nc.dram_tensor("xst", [128, N_STRIPS, IW], f16, kind="ExternalInput")
    bands = nc.dram_tensor("bands", [128, KW * MB], f16, kind="ExternalInput")
    biasv = nc.dram_tensor("biasv", [128, 1], f32, kind="ExternalInput")
    outt = nc.dram_tensor("outt", [STRIP, N_STRIPS, CW], f16, kind="ExternalOutput")

    assert sum(OUT_BATCHES) == N_STRIPS

    with TileContext(nc) as tc:
        with (
            tc.tile_pool(name="const", bufs=1) as cpool,
            tc.tile_pool(name="rhs", bufs=1) as rpool,
            tc.tile_pool(name="obuf", bufs=3) as opool,
            tc.tile_pool(name="psum", bufs=8, space="PSUM") as ppool,
        ):
            # PE clock warmup: junk matmuls keep the PE continuously busy from
            # t=0 so the p-state ramp (0.65 -> 1.2 -> 2.4 GHz after 3us of
            # continuous execution) completes while the first input chunk DMAs.
            warm_t = cpool.tile([128, MB + CW], f16)
            nc.vector.memset(warm_t[:, :], 0.0)
            wps = ppool.tile([128, CW], f32, name="wps", tag="ps")
            for _ in range(N_WARM):
                nc.tensor.matmul(
                    wps[:, :],
                    warm_t[:, :MB],
                    warm_t[:, MB : MB + CW],
                    start=True,
                    stop=True,
                )

            band_t = cpool.tile([128, KW * MB], f16)
            nc.scalar.dma_start(out=band_t[:, :], in_=bands[:, :])
            bias_t = cpool.tile([128, 1], f32)
            nc.scalar.dma_start(out=bias_t[:, :], in_=biasv[:, :])

            chunk_map = {}
            for ci, (s0, ns) in enumerate(_chunks()):
                ct = rpool.tile([128, ns * IW], f16, tag=f"rhs{ci}")
                nc.sync.dma_start(
                    out=ct[:, : ns * IW], in_=xst[:, s0 : s0 + ns, :]
                )
                for i in range(ns):
                    chunk_map[s0 + i] = (ct, i * IW)

            # strip -> (output batch tile, column offset, batch start, size)
            obuf_map = {}
            b0 = 0
            for nb in OUT_BATCHES:
                ot = opool.tile([128, max(OUT_BATCHES) * CW], f16, name="ot", tag="ot")
                for i in range(nb):
                    obuf_map[b0 + i] = (ot, i * CW, b0, nb)
                b0 += nb

            for g0 in range(0, N_STRIPS, G):
                strips = list(range(g0, min(g0 + G, N_STRIPS)))
                ps_ts, dims = [], []
                for s in strips:
                    r0 = s * STRIP
                    K = min(128, H - r0)
                    dims.append(K)
                    ps_ts.append(ppool.tile([128, CW], f32, name="ps", tag="ps"))
                for dj in range(KW):
                    lhsT = band_t[:, dj * MB : dj * MB + MB]
                    for s, ps, K in zip(strips, ps_ts, dims):
                        ct, off = chunk_map[s]
                        nc.tensor.matmul(
                            ps[:, :],
                            lhsT[:K, :],
                            ct[:K, off + dj : off + dj + CW],
                            start=(dj == 0),
                            stop=(dj == KW - 1),
                        )
                for s, ps in zip(strips, ps_ts):
                    ot, coff, ob0, nb = obuf_map[s]
                    nc.vector.tensor_scalar_add(
                        ot[:STRIP, coff : coff + CW],
                        ps[:STRIP, :],
                        bias_t[:STRIP, :1],
                    )
                    if s == ob0 + nb - 1:  # batch complete -> flush
                        nc.gpsimd.dma_start(
                            out=outt[:, ob0 : ob0 + nb, :],
                            in_=ot[:STRIP, : nb * CW],
                        )

    nc.finalize()
    return nc


def _get_nc():
    if "nc" not in _cache:
        _cache["nc"] = _build_nc()
    return _cache["nc"]


def _build_bands(weight: np.ndarray) -> np.ndarray:
    """bands[k, dj*MB + m] = weight[k - m, dj] for 0 <= k-m < KH, m < STRIP."""
    w = np.asarray(weight, np.float32)
    bands = np.zeros((128, KW * MB), np.float32)
    m = np.arange(STRIP)
    for dj in range(KW):
        for di in range(KH):
            bands[m + di, dj * MB + m] = w[di, dj]
    return bands.astype(np.float16)


def _prepare_in_maps(x, weight, bias):
    x16 = np.asarray(x, np.float32).astype(np.float16)
    bands = _build_bands(weight)
    bias_tile = np.full((128, 1), np.float32(np.asarray(bias).reshape(-1)[0]))

    # padded copy: rows up to 122*33+127, cols up to 7*512+517
    rmax = STRIP * (N_STRIPS - 1) + 128
    cmax = CW * (N_CORES - 1) + IW
    xp = np.zeros((rmax, cmax), np.float16)
    xp[:H, :W] = x16
    rows = STRIP * np.arange(N_STRIPS)[None, :] + np.arange(128)[:, None]  # [128, S]

    in_maps = []
    for c in range(N_CORES):
        c0 = c * CW
        blk = xp[:, c0 : c0 + IW]          # [rmax, IW]
        xst = np.ascontiguousarray(blk[rows])  # [128, S, IW]
        in_maps.append({"xst": xst, "bands": bands, "biasv": bias_tile})
    return in_maps


def _gather_out(per_core_outs) -> np.ndarray:
    out = np.empty((OH, OW), np.float32)
    for c in range(N_CORES):
        c0 = c * CW
        take = min(CW, OW - c0)
        ot = per_core_outs[c]["outt"]  # [STRIP, S, CW] fp16
        rows = ot.transpose(1, 0, 2).reshape(N_STRIPS * STRIP, CW)[:OH]
        out[:, c0 : c0 + take] = rows[:, :take].astype(np.float32)
    return out


def kernel(x: np.ndarray, weight: np.ndarray, bias: np.ndarray) -> np.ndarray:
    from concourse import bass_utils

    nc = _get_nc()
    in_maps = _prepare_in_maps(x, weight, bias)
    res = bass_utils.run_bass_kernel_spmd(nc, in_maps, list(range(N_CORES)))
    _cache["last_results"] = res
    return _gather_out(res.results)


# revision 17
# speedup vs baseline: 1.0641x; 1.0641x over previous
"""Trainium2 Bass kernel: 7x7 valid cross-correlation + bias on a 4096x4096 f32 image.

Formulation: banded matmul on the TensorEngine.
  out[r, c] = sum_{di,dj} w[di,dj] * x[r+di, c+dj]
For an output row-strip of M=122 rows starting at r0, using K=128 input rows:
  out[r0+m, c] = sum_k A_dj[k, m] * x[r0+k, c+dj]   summed over dj=0..6
where A_dj[k, m] = w[k-m, dj] for 0 <= k-m < 7 (a banded [128, 122] matrix,
precomputed on host from the 49 kernel weights). The 7 dj-terms accumulate
into one PSUM bank via shifted column slices of the same SBUF rhs tile.

All matmul operands are fp16 (1 cycle/row on the PE vs fp32's 4), PSUM
accumulates fp32, output staged to SBUF as fp16 and upcast on the host.

DMA strategy: each dma_start costs ~1.2us of sequencer time regardless of
size, and PE stalls reset the p-state ramp (2.4GHz only after 3us of
continuous execution). So the host pre-arranges the input strip-major as
xst[p, s, c] = x[122*s + p, c0 + c] and the kernel loads it in a few large
chunk DMAs on the SP queue; outputs are written strip-major to
outt[m, s, c] = out[122*s + m, c0 + c] in one batched DMA per weight-group
on the Activation queue. The PE then streams matmuls back-to-back.

Sharding: output columns are split across the 8 cores (512 cols/core);
each core processes all 4090 output rows. Kernel + bias replicated.
"""

import numpy as np

H, W = 4096, 4096
KH, KW = 7, 7
OH, OW = H - KH + 1, W - KW + 1  # 4090, 4090
N_CORES = 8
CW = 512               # output columns per core
IW = CW + KW - 1       # input columns per core (518)
STRIP = 122            # output rows per strip (K = STRIP + KH - 1 = 128)
MB = 128               # stationary block columns (M padded 122 -> 128)
N_STRIPS = (OH + STRIP - 1) // STRIP  # 34 (last strip M=64, K=70)
G = 4                  # strips per weight-group (PSUM pipelining)
CHUNK_SIZES = (4, 6, 8, 8, 8)  # input DMA chunks; chunk0 = exactly group0
OUT_BATCHES = (8, 8, 8, 6, 2, 1, 1)  # strips per output DMA: big early, tiny at the end
N_WARM = 8             # junk matmuls at t=0 to ramp the PE p-state clock

_cache = {}


def _chunks():
    s0 = 0
    out = []
    for ns in CHUNK_SIZES:
        ns = min(ns, N_STRIPS - s0)
        if ns <= 0:
            break
        out.append((s0, ns))
        s0 += ns
    assert s0 == N_STRIPS, (s0, N_STRIPS)
    return out


def _build_nc():
    import concourse.bacc as bacc
    import concourse.mybir as mybir
    from concourse.tile import TileContext

    f16 = mybir.dt.float16
    f32 = mybir.dt.float32

    n_chunks = len(_chunks())

    nc = bacc.Bacc("TRN2", target_bir_lowering=False, debug=False)
    xst = nc.dram_tensor("xst", [128, N_STRIPS, IW], f16, kind="ExternalInput")
    bands = nc.dram_tensor("bands", [128, KW * MB], f16, kind="ExternalInput")
    biasv = nc.dram_tensor("biasv", [128, 1], f32, kind="ExternalInput")
    outt = nc.dram_tensor("outt", [STRIP, N_STRIPS, CW], f16, kind="ExternalOutput")

    assert sum(OUT_BATCHES) == N_STRIPS

    with TileContext(nc) as tc:
        with (
            tc.tile_pool(name="const", bufs=1) as cpool,
            tc.tile_pool(name="rhs", bufs=1) as rpool,
            tc.tile_pool(name="obuf", bufs=4) as opool,
            tc.tile_pool(name="psum", bufs=8, space="PSUM") as ppool,
        ):
            # PE clock warmup: junk matmuls keep the PE continuously busy from
            # t=0 so the p-state ramp (0.65 -> 1.2 -> 2.4 GHz after 3us of
            # continuous execution) completes while the first input chunk DMAs.
            warm_t = cpool.tile([128, MB + CW], f16)
            nc.vector.memset(warm_t[:, :], 0.0)
            wps = ppool.tile([128, CW], f32, name="wps", tag="ps")
            for _ in range(N_WARM):
                nc.tensor.matmul(
                    wps[:, :],
                    warm_t[:, :MB],
                    warm_t[:, MB : MB + CW],
                    start=True,
                    stop=True,
                )

            band_t = cpool.tile([128, KW * MB], f16)
            nc.scalar.dma_start(out=band_t[:, :], in_=bands[:, :])
            bias_t = cpool.tile([128, 1], f32)
            nc.scalar.dma_start(out=bias_t[:, :], in_=biasv[:, :])

            chunk_map = {}
            for ci, (s0, ns) in enumerate(_chunks()):
                ct = rpool.tile([128, ns * IW], f16, tag=f"rhs{ci}")
                nc.sync.dma_start(
                    out=ct[:, : ns * IW], in_=xst[:, s0 : s0 + ns, :]
                )
                for i in range(ns):
                    chunk_map[s0 + i] = (ct, i * IW)

            # strip -> (output batch tile, column offset, batch start, size)
            obuf_map = {}
            b0 = 0
            for bi, nb in enumerate(OUT_BATCHES):
                if nb >= 4:
                    ot = opool.tile(
                        [128, max(OUT_BATCHES) * CW], f16, name="ot", tag="ot"
                    )
                else:
                    # tail batches get dedicated tiles: no WAR wait on a slow
                    # prior write draining from the rotation slots
                    ot = cpool.tile([128, nb * CW], f16, name=f"ot_tail{bi}")
                for i in range(nb):
                    obuf_map[b0 + i] = (ot, i * CW, b0, nb)
                b0 += nb

            for g0 in range(0, N_STRIPS, G):
                strips = list(range(g0, min(g0 + G, N_STRIPS)))
                ps_ts, dims = [], []
                for s in strips:
                    r0 = s * STRIP
                    K = min(128, H - r0)
                    dims.append(K)
                    ps_ts.append(ppool.tile([128, CW], f32, name="ps", tag="ps"))
                for dj in range(KW):
                    lhsT = band_t[:, dj * MB : dj * MB + MB]
                    for s, ps, K in zip(strips, ps_ts, dims):
                        ct, off = chunk_map[s]
                        nc.tensor.matmul(
                            ps[:, :],
                            lhsT[:K, :],
                            ct[:K, off + dj : off + dj + CW],
                            start=(dj == 0),
                            stop=(dj == KW - 1),
                        )
                for s, ps in zip(strips, ps_ts):
                    ot, coff, ob0, nb = obuf_map[s]
                    nc.vector.tensor_scalar_add(
                        ot[:STRIP, coff : coff + CW],
                        ps[:STRIP, :],
                        bias_t[:STRIP, :1],
                    )
                    if s == ob0 + nb - 1:  # batch complete -> flush
                        nc.gpsimd.dma_start(
                            out=outt[:, ob0 : ob0 + nb, :],
                            in_=ot[:STRIP, : nb * CW],
                        )

    nc.finalize()
    return nc


def _get_nc():
    if "nc" not in _cache:
        _cache["nc"] = _build_nc()
    return _cache["nc"]


def _build_bands(weight: np.ndarray) -> np.ndarray:
    """bands[k, dj*MB + m] = weight[k - m, dj] for 0 <= k-m < KH, m < STRIP."""
    w = np.asarray(weight, np.float32)
    bands = np.zeros((128, KW * MB), np.float32)
    m = np.arange(STRIP)
    for dj in range(KW):
        for di in range(KH):
            bands[m + di, dj * MB + m] = w[di, dj]
    return bands.astype(np.float16)


def _prepare_in_maps(x, weight, bias):
    x16 = np.asarray(x, np.float32).astype(np.float16)
    bands = _build_bands(weight)
    bias_tile = np.full((128, 1), np.float32(np.asarray(bias).reshape(-1)[0]))

    # padded copy: rows up to 122*33+127, cols up to 7*512+517
    rmax = STRIP * (N_STRIPS - 1) + 128
    cmax = CW * (N_CORES - 1) + IW
    xp = np.zeros((rmax, cmax), np.float16)
    xp[:H, :W] = x16
    rows = STRIP * np.arange(N_STRIPS)[None, :] + np.arange(128)[:, None]  # [128, S]

    in_maps = []
    for c in range(N_CORES):
        c0 = c * CW
        blk = xp[:, c0 : c0 + IW]          # [rmax, IW]
        xst = np.ascontiguousarray(blk[rows])  # [128, S, IW]
        in_maps.append({"xst": xst, "bands": bands, "biasv": bias_tile})
    return in_maps


def _gather_out(per_core_outs) -> np.ndarray:
    out = np.empty((OH, OW), np.float32)
    for c in range(N_CORES):
        c0 = c * CW
        take = min(CW, OW - c0)
        ot = per_core_outs[c]["outt"]  # [STRIP, S, CW] fp16
        rows = ot.transpose(1, 0, 2).reshape(N_STRIPS * STRIP, CW)[:OH]
        out[:, c0 : c0 + take] = rows[:, :take].astype(np.float32)
    return out


def kernel(x: np.ndarray, weight: np.ndarray, bias: np.ndarray) -> np.ndarray:
    from concourse import bass_utils

    nc = _get_nc()
    in_maps = _prepare_in_maps(x, weight, bias)
    res = bass_utils.run_bass_kernel_spmd(nc, in_maps, list(range(N_CORES)))
    _cache["last_results"] = res
    return _gather_out(res.results)


# revision 18
# speedup vs baseline: 1.2073x; 1.1346x over previous
"""Trainium2 Bass kernel: 7x7 valid cross-correlation + bias on a 4096x4096 f32 image.

Formulation: banded matmul on the TensorEngine.
  out[r, c] = sum_{di,dj} w[di,dj] * x[r+di, c+dj]
For an output row-strip of M=122 rows starting at r0, using K=128 input rows:
  out[r0+m, c] = sum_k A_dj[k, m] * x[r0+k, c+dj]   summed over dj=0..6
where A_dj[k, m] = w[k-m, dj] for 0 <= k-m < 7 (a banded [128, 122] matrix,
precomputed on host from the 49 kernel weights). The 7 dj-terms accumulate
into one PSUM bank via shifted column slices of the same SBUF rhs tile.

All matmul operands are fp16 (1 cycle/row on the PE vs fp32's 4), PSUM
accumulates fp32, output staged to SBUF as fp16 and upcast on the host.

DMA strategy: each dma_start costs ~1.2us of sequencer time regardless of
size, and PE stalls reset the p-state ramp (2.4GHz only after 3us of
continuous execution). So the host pre-arranges the input strip-major as
xst[p, s, c] = x[122*s + p, c0 + c] and the kernel loads it in a few large
chunk DMAs on the SP queue; outputs are written strip-major to
outt[m, s, c] = out[122*s + m, c0 + c] in one batched DMA per weight-group
on the Activation queue. The PE then streams matmuls back-to-back.

Sharding: output columns are split across the 8 cores (512 cols/core);
each core processes all 4090 output rows. Kernel + bias replicated.
"""

import numpy as np

H, W = 4096, 4096
KH, KW = 7, 7
OH, OW = H - KH + 1, W - KW + 1  # 4090, 4090
N_CORES = 8
CW = 512               # output columns per core
IW = CW + KW - 1       # input columns per core (518)
STRIP = 122            # output rows per strip (K = STRIP + KH - 1 = 128)
MB = 128               # stationary block columns (M padded 122 -> 128)
N_STRIPS = (OH + STRIP - 1) // STRIP  # 34 (last strip M=64, K=70)
G = 4                  # strips per weight-group (PSUM pipelining)
CHUNK_SIZES = (4, 6, 8, 8, 8)  # input DMA chunks; chunk0 = exactly group0
OUT_BATCHES = (8, 8, 8, 6, 2, 1, 1)  # strips per output DMA: big early, tiny at the end
N_WARM = 8             # junk matmuls at t=0 to ramp the PE p-state clock

_cache = {}


def _chunks():
    s0 = 0
    out = []
    for ns in CHUNK_SIZES:
        ns = min(ns, N_STRIPS - s0)
        if ns <= 0:
            break
        out.append((s0, ns))
        s0 += ns
    assert s0 == N_STRIPS, (s0, N_STRIPS)
    return out


def _build_nc():
    import concourse.bacc as bacc
    import concourse.mybir as mybir
    from concourse.tile import TileContext

    f16 = mybir.dt.float16
    f32 = mybir.dt.float32

    n_chunks = len(_chunks())

    nc = bacc.Bacc("TRN2", target_bir_lowering=False, debug=False)
    xst = nc.dram_tensor("xst", [128, N_STRIPS, IW], f16, kind="ExternalInput")
    bands = nc.dram_tensor("bands", [128, KW * MB], f16, kind="ExternalInput")
    biasv = nc.dram_tensor("biasv", [128, 1], f32, kind="ExternalInput")
    outt = nc.dram_tensor("outt", [STRIP, N_STRIPS, CW], f16, kind="ExternalOutput")

    assert sum(OUT_BATCHES) == N_STRIPS

    with TileContext(nc) as tc:
        with (
            tc.tile_pool(name="const", bufs=1) as cpool,
            tc.tile_pool(name="rhs", bufs=1) as rpool,
            tc.tile_pool(name="obuf", bufs=4) as opool,
            tc.tile_pool(name="psum", bufs=8, space="PSUM") as ppool,
        ):
            # PE clock warmup: junk matmuls keep the PE continuously busy from
            # t=0 so the p-state ramp (0.65 -> 1.2 -> 2.4 GHz after 3us of
            # continuous execution) completes while the first input chunk DMAs.
            warm_t = cpool.tile([128, MB + CW], f16)
            nc.vector.memset(warm_t[:, :], 0.0)
            wps = ppool.tile([128, CW], f32, name="wps", tag="ps")
            for _ in range(N_WARM):
                nc.tensor.matmul(
                    wps[:, :],
                    warm_t[:, :MB],
                    warm_t[:, MB : MB + CW],
                    start=True,
                    stop=True,
                )

            band_t = cpool.tile([128, KW * MB], f16)
            nc.scalar.dma_start(out=band_t[:, :], in_=bands[:, :])
            bias_t = cpool.tile([128, 1], f32)
            nc.scalar.dma_start(out=bias_t[:, :], in_=biasv[:, :])

            chunk_map = {}
            for ci, (s0, ns) in enumerate(_chunks()):
                ct = rpool.tile([128, ns * IW], f16, tag=f"rhs{ci}")
                nc.sync.dma_start(
                    out=ct[:, : ns * IW], in_=xst[:, s0 : s0 + ns, :]
                )
                for i in range(ns):
                    chunk_map[s0 + i] = (ct, i * IW)

            # strip -> (output batch tile, column offset, batch start, size)
            obuf_map = {}
            b0 = 0
            for bi, nb in enumerate(OUT_BATCHES):
                if nb >= 4:
                    ot = opool.tile(
                        [128, max(OUT_BATCHES) * CW], f16, name="ot", tag="ot"
                    )
                else:
                    # tail batches get dedicated tiles: no WAR wait on a slow
                    # prior write draining from the rotation slots
                    ot = cpool.tile([128, nb * CW], f16, name=f"ot_tail{bi}")
                for i in range(nb):
                    obuf_map[b0 + i] = (ot, i * CW, b0, nb)
                b0 += nb

            for g0 in range(0, N_STRIPS, G):
                strips = list(range(g0, min(g0 + G, N_STRIPS)))
                ps_ts, dims = [], []
                for s in strips:
                    r0 = s * STRIP
                    K = min(128, H - r0)
                    dims.append(K)
                    ps_ts.append(ppool.tile([128, CW], f32, name="ps", tag="ps"))
                for dj in range(KW):
                    lhsT = band_t[:, dj * MB : dj * MB + MB]
                    for s, ps, K in zip(strips, ps_ts, dims):
                        ct, off = chunk_map[s]
                        nc.tensor.matmul(
                            ps[:, :],
                            lhsT[:K, :],
                            ct[:K, off + dj : off + dj + CW],
                            start=(dj == 0),
                            stop=(dj == KW - 1),
                        )
                for s, ps in zip(strips, ps_ts):
                    ot, coff, ob0, nb = obuf_map[s]
                    nc.vector.tensor_scalar_add(
                        ot[:STRIP, coff : coff + CW],
                        ps[:STRIP, :],
                        bias_t[:STRIP, :1],
                    )
                    if s == ob0 + nb - 1:  # batch complete -> flush
                        # split the write between the SWDGE path (spreads
                        # across all 16 DMA engines, ~90GB/s) and the HWDGE
                        # path (pinned to 2 engines, ~45GB/s) to add rates
                        k = (2 * nb + 2) // 3
                        nc.gpsimd.dma_start(
                            out=outt[:, ob0 : ob0 + k, :],
                            in_=ot[:STRIP, : k * CW],
                        )
                        if k < nb:
                            nc.scalar.dma_start(
                                out=outt[:, ob0 + k : ob0 + nb, :],
                                in_=ot[:STRIP, k * CW : nb * CW],
                            )

    nc.finalize()
    return nc


def _get_nc():
    if "nc" not in _cache:
        _cache["nc"] = _build_nc()
    return _cache["nc"]


def _build_bands(weight: np.ndarray) -> np.ndarray:
    """bands[k, dj*MB + m] = weight[k - m, dj] for 0 <= k-m < KH, m < STRIP."""
    w = np.asarray(weight, np.float32)
    bands = np.zeros((128, KW * MB), np.float32)
    m = np.arange(STRIP)
    for dj in range(KW):
        for di in range(KH):
            bands[m + di, dj * MB + m] = w[di, dj]
    return bands.astype(np.float16)


def _prepare_in_maps(x, weight, bias):
    x16 = np.asarray(x, np.float32).astype(np.float16)
    bands = _build_bands(weight)
    bias_tile = np.full((128, 1), np.float32(np.asarray(bias).reshape(-1)[0]))

    # padded copy: rows up to 122*33+127, cols up to 7*512+517
    rmax = STRIP * (N_STRIPS - 1) + 128
    cmax = CW * (N_CORES - 1) + IW
    xp = np.zeros((rmax, cmax), np.float16)
    xp[:H, :W] = x16
    rows = STRIP * np.arange(N_STRIPS)[None, :] + np.arange(128)[:, None]  # [128, S]

    in_maps = []
    for c in range(N_CORES):
        c0 = c * CW
        blk = xp[:, c0 : c0 + IW]          # [rmax, IW]
        xst = np.ascontiguousarray(blk[rows])  # [128, S, IW]
        in_maps.append({"xst": xst, "bands": bands, "biasv": bias_tile})
    return in_maps


def _gather_out(per_core_outs) -> np.ndarray:
    out = np.empty((OH, OW), np.float32)
    for c in range(N_CORES):
        c0 = c * CW
        take = min(CW, OW - c0)
        ot = per_core_outs[c]["outt"]  # [STRIP, S, CW] fp16
        rows = ot.transpose(1, 0, 2).reshape(N_STRIPS * STRIP, CW)[:OH]
        out[:, c0 : c0 + take] = rows[:, :take].astype(np.float32)
    return out


def kernel(x: np.ndarray, weight: np.ndarray, bias: np.ndarray) -> np.ndarray:
    from concourse import bass_utils

    nc = _get_nc()
    in_maps = _prepare_in_maps(x, weight, bias)
    res = bass_utils.run_bass_kernel_spmd(nc, in_maps, list(range(N_CORES)))
    _cache["last_results"] = res
    return _gather_out(res.results)
